# revision 6
# baseline (speedup 1.0000x reference)
"""SkeletalPool Trainium2 kernel.

Computes out = (x[:, IDX0] + x[:, IDX1]) * 0.5 for the skeletal pooling
map: joint 0 passes through, joints (2i-1, 2i) average into output joint
i (i = 1..15).

  x:   [32, 31, 64, 4096] f32
  out: [32, 16, 64, 4096] f32

Sharding: pure data parallelism over batch - 32 batches / 8 cores = 4
per core, no communication.

Per-core kernel ('split2gp' schedule): both HWDGE DMA rings (SP and ACT)
carry an interleaved mix of loads and stores - a single HWDGE ring
saturates at ~240 GB/s, well under the ~358 GB/s per-core HBM limit, so
dedicating one ring to loads (2x the bytes of stores) leaves the load
ring as the bottleneck; balancing bytes across both rings measured ~17%
faster end to end, and moving the root-joint DRAM->DRAM copies to the
gpsimd SWDGE queue (a third, otherwise idle DMA queue) another ~18%.
DVE does the adds and the x0.5 scale (exact), so the ACT engine issues
DMAs only.

Work unit = 3 pairs (6 input joints, 6 MiB in; 3 output joints, 3 MiB
out) viewed as [128 partitions, joints, 2048] (partition = (channel,
half-row): each contiguous 1 MiB joint block [64, 4096] reinterpreted as
[128, 2048]). Loads for even units issue on the SP ring, odd units on
the ACT ring; each unit's store issues on the opposite ring, lagging one
unit behind that ring's loads. Double-buffered SBUF slots; every wait is
a standalone sequencer wait_ge (this walrus build rejects DMAs carrying
more than one sync-wait), and per-slot DMA semaphores keep counts exact:
slot reuse is gated through the compute chain, so same-slot DMAs are
ordered and a count of 16n implies the first n completed.
"""

import sys

if "/opt/trn_rl_repo" not in sys.path:
    sys.path.insert(0, "/opt/trn_rl_repo")

import numpy as np

import concourse.bass as bass
import concourse.mybir as mybir
from concourse.bass_utils import run_bass_kernel_spmd

N_CORES = 8
B_FULL = 32
B_SHARD = B_FULL // N_CORES  # 4
J_IN = 31
J_OUT = 16
C = 64
T = 4096
P = 128
TT = (C * T) // P  # 2048
PAIRS = 3  # pairs per unit
JC = 2 * PAIRS  # 6 input joints per unit
N_CHUNKS = 15 // PAIRS  # 5 units per batch
N_TASKS = B_SHARD * N_CHUNKS  # 20 units per core
NBUF = 2

f32 = mybir.dt.float32

_CACHE = {}


def _build_nc(reps: int = 1) -> bass.Bass:
    nc = bass.Bass("TRN2", debug=False, num_devices=N_CORES)
    x = nc.dram_tensor("x", (B_SHARD, J_IN, C, T), f32, kind="ExternalInput")
    out = nc.dram_tensor("out", (B_SHARD, J_OUT, C, T), f32, kind="ExternalOutput")
    xp = x.ap().rearrange("b j c (u t) -> b (c u) j t", u=2)  # [4,128,31,2048]
    op = out.ap().rearrange("b j c (u t) -> b (c u) j t", u=2)  # [4,128,16,2048]

    tin = nc.alloc_sbuf_tensor("tin", [P, NBUF * JC * TT], f32)
    tout = nc.alloc_sbuf_tensor("tout", [P, NBUF * PAIRS * TT], f32)
    s_load = [nc.alloc_semaphore(f"s_load{i}") for i in range(NBUF)]
    s_store = [nc.alloc_semaphore(f"s_store{i}") for i in range(NBUF)]
    s_add = nc.alloc_semaphore("s_add")
    s_mul = nc.alloc_semaphore("s_mul")
    s_copy = nc.alloc_semaphore("s_copy")

    TOT = reps * N_TASKS

    def task(g):
        b, chunk = divmod(g % N_TASKS, N_CHUNKS)
        return b, 1 + chunk * JC, 1 + chunk * PAIRS

    def tin_v(g):  # [128, 6, 2048]
        s = (g % NBUF) * JC * TT
        return tin.ap()[:, s : s + JC * TT].rearrange("p (j t) -> p j t", j=JC)

    def tout_slot(g):  # [128, 3*2048]
        s = (g % NBUF) * PAIRS * TT
        return tout.ap()[:, s : s + PAIRS * TT]

    def issue_load(eng, g):
        b, jin, _ = task(g)
        if g >= NBUF:
            # tin slot free once task g-NBUF's adds are done
            eng.wait_ge(s_add, PAIRS * (g - NBUF + 1))
        eng.dma_start(out=tin_v(g), in_=xp[b, :, jin : jin + JC, :]).then_inc(
            s_load[g % NBUF], 16
        )

    def issue_store(eng, g):
        b, _, jout = task(g)
        eng.wait_ge(s_mul, g + 1)
        eng.dma_start(
            out=op[b, :, jout : jout + PAIRS, :],
            in_=tout_slot(g).rearrange("p (j t) -> p j t", j=PAIRS),
        ).then_inc(s_store[g % NBUF], 16)

    def issue_copy(eng, g):
        b, _, _ = task(g)
        # root joint: (x0 + x0) * 0.5 == x0 exactly -> straight DRAM->DRAM
        eng.dma_start(out=op[b, :, 0, :], in_=xp[b, :, 0, :]).then_inc(s_copy, 16)

    n_copies = reps * B_SHARD

    def ring_prog(eng, r):
        # loads for units g = r (mod 2); store(g-1) after load(g) so the
        # store trails this ring's own loads.
        for g in range(TOT):
            if g % 2 != r:
                continue
            issue_load(eng, g)
            if g >= 1:
                issue_store(eng, g - 1)
        if (TOT - 1) % 2 != r:
            issue_store(eng, TOT - 1)
        # gate kernel end on the stores this ring issued (slot parity 1-r)
        eng.wait_ge(s_store[1 - r], 16 * (TOT // NBUF))
        if r == 0:
            eng.wait_ge(s_copy, 16 * n_copies)

    with nc.Block() as block:

        @block.sync
        def _(sync):
            ring_prog(sync, 0)

        @block.scalar
        def _(scalar):
            ring_prog(scalar, 1)

        @block.gpsimd
        def _(gp):
            # root copies on the SWDGE queue: keeps the 8 MiB of D2D
            # traffic (read + write per copy) off the two HWDGE rings,
            # measured ~18% faster end to end.
            for rix in range(reps):
                for b in range(B_SHARD):
                    issue_copy(gp, rix * N_TASKS + b * N_CHUNKS)

        @block.vector
        def _(vector):
            for g in range(TOT):
                vector.wait_ge(s_load[g % NBUF], 16 * (g // NBUF + 1))
                if g >= NBUF:
                    # tout slot free once its previous store completed
                    vector.wait_ge(s_store[g % NBUF], 16 * (g // NBUF))
                    vector.wait_ge(s_mul, g - 1)
                tv, ov = tin_v(g), tout_slot(g)
                for i in range(PAIRS):
                    vector.tensor_add(
                        out=ov[:, i * TT : (i + 1) * TT],
                        in0=tv[:, 2 * i, :],
                        in1=tv[:, 2 * i + 1, :],
                    ).then_inc(s_add, 1)
                vector.wait_ge(s_add, PAIRS * (g + 1))
                # halving is exact: add-then-scale == (a + b) * 0.5 bitwise
                vector.tensor_scalar_mul(ov, ov, 0.5).then_inc(s_mul, 1)

    return nc


def get_nc() -> bass.Bass:
    if "nc" not in _CACHE:
        _CACHE["nc"] = _build_nc(1)
    return _CACHE["nc"]


def kernel(x: np.ndarray, **run_kwargs):
    x = np.ascontiguousarray(np.asarray(x, dtype=np.float32))
    assert x.shape == (B_FULL, J_IN, C, T), x.shape

    nc = get_nc()
    in_maps = [
        {"x": np.ascontiguousarray(x[i * B_SHARD : (i + 1) * B_SHARD])}
        for i in range(N_CORES)
    ]
    res = run_bass_kernel_spmd(nc, in_maps, core_ids=list(range(N_CORES)), **run_kwargs)
    out = np.concatenate([res.results[i]["out"] for i in range(N_CORES)], axis=0)
    _CACHE["last_results"] = res
    return out


# revision 7
# speedup vs baseline: 1.5016x; 1.5016x over previous
"""SkeletalPool Trainium2 kernel.

Computes out = (x[:, IDX0] + x[:, IDX1]) * 0.5 for the skeletal pooling
map: joint 0 passes through, joints (2i-1, 2i) average into output joint
i (i = 1..15).

  x:   [32, 31, 64, 4096] f32
  out: [32, 16, 64, 4096] f32

Sharding: pure data parallelism over batch - 32 batches / 8 cores = 4
per core, no communication.

Per-core kernel ('split2' schedule): both HWDGE DMA rings (SP and ACT)
carry an interleaved, byte-balanced mix of loads and stores (a single
HWDGE ring saturates at ~240 GB/s, under the ~358 GB/s per-core HBM
limit, so the baseline's dedicated load ring was the structural
bottleneck). DVE does the adds and the x0.5 scale (exact), so the ACT
engine issues DMAs only. Root joints are DRAM->DRAM copies on the ring
matching the batch parity. The kernel uses no gpsimd instructions and
skips the expensive per-execution gpsimd dge_drain at block exit
(no_gpsimd_drain=True) to cut per-call fixed overhead.

Work unit = 3 pairs (6 input joints, 6 MiB in; 3 output joints, 3 MiB
out) viewed as [128 partitions, joints, 2048] (partition = (channel,
half-row): each contiguous 1 MiB joint block [64, 4096] reinterpreted as
[128, 2048]). Loads for even units issue on the SP ring, odd units on
the ACT ring; each unit's store issues on the opposite ring, lagging one
unit behind that ring's loads. Double-buffered SBUF slots; every wait is
a standalone sequencer wait_ge (this walrus build rejects DMAs carrying
more than one sync-wait), and per-slot DMA semaphores keep counts exact:
slot reuse is gated through the compute chain, so same-slot DMAs are
ordered and a count of 16n implies the first n completed.
"""

import sys

if "/opt/trn_rl_repo" not in sys.path:
    sys.path.insert(0, "/opt/trn_rl_repo")

import numpy as np

import concourse.bass as bass
import concourse.mybir as mybir
from concourse.bass_utils import run_bass_kernel_spmd

N_CORES = 8
B_FULL = 32
B_SHARD = B_FULL // N_CORES  # 4
J_IN = 31
J_OUT = 16
C = 64
T = 4096
P = 128
TT = (C * T) // P  # 2048
PAIRS = 3  # pairs per unit
JC = 2 * PAIRS  # 6 input joints per unit
N_CHUNKS = 15 // PAIRS  # 5 units per batch
N_TASKS = B_SHARD * N_CHUNKS  # 20 units per core
NBUF = 2

f32 = mybir.dt.float32

_CACHE = {}


def _build_nc(reps: int = 1) -> bass.Bass:
    nc = bass.Bass("TRN2", debug=False, num_devices=N_CORES)
    x = nc.dram_tensor("x", (B_SHARD, J_IN, C, T), f32, kind="ExternalInput")
    out = nc.dram_tensor("out", (B_SHARD, J_OUT, C, T), f32, kind="ExternalOutput")
    xp = x.ap().rearrange("b j c (u t) -> b (c u) j t", u=2)  # [4,128,31,2048]
    op = out.ap().rearrange("b j c (u t) -> b (c u) j t", u=2)  # [4,128,16,2048]

    tin = nc.alloc_sbuf_tensor("tin", [P, NBUF * JC * TT], f32)
    tout = nc.alloc_sbuf_tensor("tout", [P, NBUF * PAIRS * TT], f32)
    s_load = [nc.alloc_semaphore(f"s_load{i}") for i in range(NBUF)]
    s_store = [nc.alloc_semaphore(f"s_store{i}") for i in range(NBUF)]
    s_add = nc.alloc_semaphore("s_add")
    s_mul = nc.alloc_semaphore("s_mul")
    s_copy = nc.alloc_semaphore("s_copy")

    TOT = reps * N_TASKS

    def task(g):
        b, chunk = divmod(g % N_TASKS, N_CHUNKS)
        return b, 1 + chunk * JC, 1 + chunk * PAIRS

    def tin_v(g):  # [128, 6, 2048]
        s = (g % NBUF) * JC * TT
        return tin.ap()[:, s : s + JC * TT].rearrange("p (j t) -> p j t", j=JC)

    def tout_slot(g):  # [128, 3*2048]
        s = (g % NBUF) * PAIRS * TT
        return tout.ap()[:, s : s + PAIRS * TT]

    def issue_load(eng, g):
        b, jin, _ = task(g)
        if g >= NBUF:
            # tin slot free once task g-NBUF's adds are done
            eng.wait_ge(s_add, PAIRS * (g - NBUF + 1))
        eng.dma_start(out=tin_v(g), in_=xp[b, :, jin : jin + JC, :]).then_inc(
            s_load[g % NBUF], 16
        )

    def issue_store(eng, g):
        b, _, jout = task(g)
        eng.wait_ge(s_mul, g + 1)
        eng.dma_start(
            out=op[b, :, jout : jout + PAIRS, :],
            in_=tout_slot(g).rearrange("p (j t) -> p j t", j=PAIRS),
        ).then_inc(s_store[g % NBUF], 16)

    def issue_copy(eng, g):
        b, _, _ = task(g)
        # root joint: (x0 + x0) * 0.5 == x0 exactly -> straight DRAM->DRAM
        eng.dma_start(out=op[b, :, 0, :], in_=xp[b, :, 0, :]).then_inc(s_copy, 16)

    n_copies = reps * B_SHARD

    def ring_prog(eng, r):
        # loads for units g = r (mod 2); store(g-1) after load(g) so the
        # store trails this ring's own loads.
        for g in range(TOT):
            if g % 2 != r:
                continue
            # unit g = 5*(4*rep + b) + k, so k == 0 implies g's parity is
            # b's parity: each batch's root copy lands on ring b % 2.
            if g % N_CHUNKS == 0:
                issue_copy(eng, g)
            issue_load(eng, g)
            if g >= 1:
                issue_store(eng, g - 1)
        if (TOT - 1) % 2 != r:
            issue_store(eng, TOT - 1)
        # gate kernel end on the stores this ring issued (slot parity 1-r)
        eng.wait_ge(s_store[1 - r], 16 * (TOT // NBUF))
        if r == 0:
            eng.wait_ge(s_copy, 16 * n_copies)

    # No gpsimd instructions anywhere, and the block-exit gpsimd dge_drain
    # is expensive per execution - skip it. All DMA completion is already
    # gated through semaphore waits on the SP/ACT sequencers.
    with nc.Block(no_gpsimd_drain=True) as block:

        @block.sync
        def _(sync):
            ring_prog(sync, 0)

        @block.scalar
        def _(scalar):
            ring_prog(scalar, 1)

        @block.vector
        def _(vector):
            for g in range(TOT):
                vector.wait_ge(s_load[g % NBUF], 16 * (g // NBUF + 1))
                if g >= NBUF:
                    # tout slot free once its previous store completed
                    vector.wait_ge(s_store[g % NBUF], 16 * (g // NBUF))
                    vector.wait_ge(s_mul, g - 1)
                tv, ov = tin_v(g), tout_slot(g)
                for i in range(PAIRS):
                    vector.tensor_add(
                        out=ov[:, i * TT : (i + 1) * TT],
                        in0=tv[:, 2 * i, :],
                        in1=tv[:, 2 * i + 1, :],
                    ).then_inc(s_add, 1)
                vector.wait_ge(s_add, PAIRS * (g + 1))
                # halving is exact: add-then-scale == (a + b) * 0.5 bitwise
                vector.tensor_scalar_mul(ov, ov, 0.5).then_inc(s_mul, 1)

    return nc


def get_nc() -> bass.Bass:
    if "nc" not in _CACHE:
        _CACHE["nc"] = _build_nc(1)
    return _CACHE["nc"]


def kernel(x: np.ndarray, **run_kwargs):
    x = np.ascontiguousarray(np.asarray(x, dtype=np.float32))
    assert x.shape == (B_FULL, J_IN, C, T), x.shape

    nc = get_nc()
    in_maps = [
        {"x": np.ascontiguousarray(x[i * B_SHARD : (i + 1) * B_SHARD])}
        for i in range(N_CORES)
    ]
    res = run_bass_kernel_spmd(nc, in_maps, core_ids=list(range(N_CORES)), **run_kwargs)
    out = np.concatenate([res.results[i]["out"] for i in range(N_CORES)], axis=0)
    _CACHE["last_results"] = res
    return out
